# revision 15
# baseline (speedup 1.0000x reference)
"""Trainium2 Bass kernel for the DCN Cross layer:

    out = x0 * (x @ weights)[:, None] + bias + x

with x0, x: [16384, 2048] f32, weights/bias: [2048] f32.

Strategy: data-parallel over the batch dim across 8 NeuronCores
(2048 rows per core).

DMA-topology insight (measured on this hardware): SDMA engines run at
~50 GB/s each (~740 GB/s/core aggregate) when every descriptor reads
one contiguous DRAM chunk and the kernel has a single load stream plus
a single store stream; interleaving two distant read streams (separate
x0/x tensors) halves the effective rate.  So the host packs x0 and x
row-interleaved into one tensor, giving each partition a fully
contiguous 32 KB chunk per 2-tile group load:

  xx[p, n, t, f] = (x0 if t==0 else x)[shard row p*16+n, f]

Row r of the shard maps to (partition p = r // 16, tile n = r % 16);
loads and stores both use this mapping so no host-side unshuffle of
the output is needed (the math is row-independent).

Per 2-tile group (one 4 MB load, one 2 MB store):
  1. xw = reduce_add(x rows)        (DVE tensor_reduce; for
     non-uniform weights a GPSIMD multiply by a broadcast weights tile
     feeds the reduce; tensor_tensor_reduce would fuse this but it
     crashes TRN2 hardware in this runtime)
  2. out = (x0 * xw) + x (+ bias)   (scalar_tensor_tensor, in place
     into the x0 sub-tiles; split between DVE and GPSIMD so the DVE
     does not become the critical path now that DMA is fast)
"""

import os
import sys

import numpy as np


def _ensure_paths():
    for p in (
        "/root/.axon_site",
        "/root/.axon_site/_ro/trn_rl_repo",
        "/root/.axon_site/_ro/pypackages",
        "/opt/trn_rl_repo",
        "/opt/pypackages",
    ):
        if os.path.isdir(p) and p not in sys.path:
            sys.path.append(p)


_ensure_paths()

N_CORES = 8
B, F = 16384, 2048
P = 128                 # SBUF partitions
R = B // N_CORES        # rows per core (2048)
N_TILES = R // P        # 16 row-tiles per core

_NC_CACHE = {}


def _build_nc(has_bias: bool, uniform_w: bool, w0: float, offload: bool = True):
    import concourse.bacc as bacc
    import concourse.mybir as mybir
    from concourse.tile import TileContext

    f32 = mybir.dt.float32
    Alu = mybir.AluOpType

    nc = bacc.Bacc("TRN2", target_bir_lowering=False)
    xx = nc.dram_tensor("xx", [2 * R, F], f32, kind="ExternalInput")
    if not uniform_w:
        wb = nc.dram_tensor("w_bcast", [P, F], f32, kind="ExternalInput")
    if has_bias:
        bb = nc.dram_tensor("b_bcast", [P, F], f32, kind="ExternalInput")
    out = nc.dram_tensor("out", [R, F], f32, kind="ExternalOutput")

    # xx rows are laid out (p, n, t): shard row p*N_TILES+n of tensor t.
    xx_t = xx.rearrange("(p n t) f -> n p t f", p=P, t=2)
    out_t = out.rearrange("(p n) f -> n p f", p=P)

    # 2-tile groups, with the final two tiles run singly so the
    # pipeline tail (last compute + last store) is short.
    groups = []
    i = 0
    while i < N_TILES:
        g = 2 if i < N_TILES - 2 else 1
        groups.append((i, g))
        i += g
    GMAX = max(g for _, g in groups)

    with TileContext(nc) as tc:
        with (
            tc.tile_pool(name="const", bufs=1) as cpool,
            tc.tile_pool(name="work", bufs=4) as wpool,
            tc.tile_pool(name="scal", bufs=6) as spool,
        ):
            if not uniform_w:
                w_sb = cpool.tile([P, F], f32)
                nc.sync.dma_start(out=w_sb, in_=wb[:, :])
            if has_bias:
                b_sb = cpool.tile([P, F], f32)
                nc.sync.dma_start(out=b_sb, in_=bb[:, :])

            for gi, (i0, g) in enumerate(groups):
                xx_sb = wpool.tile(
                    [P, GMAX, 2, F], f32, tag="xx", name="xx_sb"
                )[:, :g, :, :]
                xw = spool.tile([P, GMAX], f32, tag="xw", name="xw")[:, :g]

                # One load: per partition a contiguous g*2*F chunk.
                nc.sync.dma_start(
                    out=xx_sb,
                    in_=xx_t[i0 : i0 + g].rearrange("j p t f -> p j t f"),
                )
                x0_v = xx_sb[:, :, 0, :]   # [P, g, F]
                x_v = xx_sb[:, :, 1, :]    # [P, g, F]

                # xw[p, j] = sum_f x[p, j, f] * w[f]
                if uniform_w:
                    reduce_src = x_v
                else:
                    tmp_sb = wpool.tile(
                        [P, GMAX, F], f32, tag="tmp", name="tmp_sb"
                    )[:, :g, :]
                    for j in range(g):
                        nc.gpsimd.tensor_tensor(
                            out=tmp_sb[:, j, :],
                            in0=x_v[:, j, :],
                            in1=w_sb,
                            op=Alu.mult,
                        )
                    reduce_src = tmp_sb
                nc.vector.tensor_reduce(
                    out=xw,
                    in_=reduce_src,
                    axis=mybir.AxisListType.X,
                    op=Alu.add,
                )
                if uniform_w and w0 != 1.0:
                    nc.vector.tensor_scalar(
                        out=xw,
                        in0=xw,
                        scalar1=float(w0),
                        scalar2=None,
                        op0=Alu.mult,
                    )

                if has_bias:
                    t_sb = wpool.tile(
                        [P, GMAX, F], f32, tag="t", name="t_sb"
                    )[:, :g, :]
                    for j in range(g):
                        nc.gpsimd.tensor_tensor(
                            out=t_sb[:, j, :],
                            in0=x_v[:, j, :],
                            in1=b_sb,
                            op=Alu.add,
                        )
                    addend = t_sb
                else:
                    addend = x_v

                # out = x0 * xw + addend, in place into the x0 sub-tiles.
                # Alternate sub-tiles between DVE and GPSIMD so neither
                # engine paces the (DMA-bound) pipeline.
                for j in range(g):
                    eng = (
                        nc.gpsimd
                        if (offload and (2 * gi + j) % 2 == 1)
                        else nc.vector
                    )
                    eng.scalar_tensor_tensor(
                        out=x0_v[:, j, :],
                        in0=x0_v[:, j, :],
                        scalar=xw[:, j : j + 1],
                        in1=addend[:, j, :],
                        op0=Alu.mult,
                        op1=Alu.add,
                    )

                # Store on the ACT HWDGE ring, keeping the load stream
                # (Sync ring) free of store waits.
                nc.scalar.dma_start(
                    out=out_t[i0 : i0 + g].rearrange("j p f -> p j f"),
                    in_=x0_v,
                )

    nc.finalize()
    return nc


def _get_nc(has_bias: bool, uniform_w: bool, w0: float):
    offload = os.environ.get("CROSS_OFFLOAD", "1") != "0"
    key = ("cross", has_bias, uniform_w, w0 if uniform_w else None, offload)
    if key not in _NC_CACHE:
        _NC_CACHE[key] = _build_nc(has_bias, uniform_w, w0, offload)
    return _NC_CACHE[key]


def _make_in_maps(x0, x, w, b, has_bias, uniform_w):
    if not uniform_w:
        wbt = np.ascontiguousarray(np.broadcast_to(w.reshape(1, F), (P, F)))
    if has_bias:
        bbt = np.ascontiguousarray(np.broadcast_to(b.reshape(1, F), (P, F)))
    in_maps = []
    for c in range(N_CORES):
        x0s = x0[c * R : (c + 1) * R].reshape(P, N_TILES, 1, F)
        xs = x[c * R : (c + 1) * R].reshape(P, N_TILES, 1, F)
        xxs = np.concatenate([x0s, xs], axis=2).reshape(2 * R, F)
        m = {"xx": np.ascontiguousarray(xxs)}
        if not uniform_w:
            m["w_bcast"] = wbt
        if has_bias:
            m["b_bcast"] = bbt
        in_maps.append(m)
    return in_maps


def run_spmd(inputs, trace=False, **kwargs):
    """Shard, run on 8 cores, gather. Returns (output, BassKernelResults)."""
    from concourse.bass_utils import run_bass_kernel_spmd

    x0 = np.asarray(inputs["x0"], dtype=np.float32)
    x = np.asarray(inputs["x"], dtype=np.float32)
    w = np.asarray(
        inputs.get("weights", np.ones((F,), np.float32)), dtype=np.float32
    )
    b = np.asarray(
        inputs.get("bias", np.zeros((F,), np.float32)), dtype=np.float32
    )
    assert x0.shape == (B, F) and x.shape == (B, F)

    has_bias = bool(np.any(b != 0.0))
    w0 = float(w.flat[0])
    uniform_w = bool(np.all(w == w0))
    nc = _get_nc(has_bias, uniform_w, w0)
    in_maps = _make_in_maps(x0, x, w, b, has_bias, uniform_w)
    res = run_bass_kernel_spmd(
        nc, in_maps, core_ids=list(range(N_CORES)), trace=trace, **kwargs
    )
    # Gather: shard row r of core c is full row c*R + r; the kernel's
    # internal (p, n) mapping is consistent between loads and stores,
    # so the output needs no unshuffle.
    out = np.concatenate(
        [res.results[c]["out"] for c in range(N_CORES)], axis=0
    )
    return out.astype(np.float32, copy=False), res


def kernel(**inputs) -> np.ndarray:
    out, _ = run_spmd(inputs, trace=False)
    return out
